# revision 8
# baseline (speedup 1.0000x reference)
"""DRGRU (diffusion-conv GRU cell) Trainium2 kernel.

Per-core (8 cores, one batch sample each):
  A0 = diag(1/colsum(adj+I)) @ (adj+I),  A1 = diag(1/colsum(adj^T+I)) @ (adj^T+I)
  gconv(x) = [x, A0 x, A0 x, A0^2 x] @ W0^T + [x, A1 x, A1 x, A1^2 x] @ W1^T + b
  value = sigmoid(gconv1(cat(xi, xh)));  r,u = split(value)
  c = tanh(gconv2(cat(xi, r*xh)));  out = u*xh + (1-u)*c

Device layout strategy: diffusion hops compute z^T = act.T @ adjI_or[j,i]
(stationary = activation node-tile, moving = un-normalized adjacency), with the
row-normalization 1/d applied as a per-output-column scale folded into the
PSUM->SBUF copy (d is computed on device via ones-vector matmuls + reciprocal +
outer-product broadcast).  Projections contract features with small stationary
weights against feat-major activations.  PE transposes flip layouts between
hops.  All matmul operands are float32r (TF32-like, full PE rate at >=256 free).
"""

import os

import numpy as np
import ml_dtypes

import concourse.bacc as bacc
import concourse.mybir as mybir
from concourse import tile
from concourse.bass_utils import run_bass_kernel_spmd

B, N, D = 8, 1024, 64
F = 2 * D       # 128 per-node features into gconv1
NT = N // 128   # 8 node tiles
O1, O2 = 2 * D, D

F32 = mybir.dt.float32

_DT_NAME = os.environ.get("DRGRU_DT", "f32r")
if _DT_NAME == "bf16":
    DT, NPDT, FREE = mybir.dt.bfloat16, ml_dtypes.bfloat16, 1024
else:
    DT, NPDT, FREE = mybir.dt.float32r, np.float32, 512
CH = [(i, min(i + FREE, N)) for i in range(0, N, FREE)]

_cache: dict = {}


def _build_nc():
    nc = bacc.Bacc("TRN2", target_bir_lowering=False, debug=False, num_devices=8)

    a0t_d = nc.declare_dram_parameter("a0t", [N, N], DT, isOutput=False)
    a1t_d = nc.declare_dram_parameter("a1t", [N, N], DT, isOutput=False)
    xi_d = nc.declare_dram_parameter("xi", [N, D], F32, isOutput=False)
    xh_d = nc.declare_dram_parameter("xh", [N, D], F32, isOutput=False)
    xit_d = nc.declare_dram_parameter("xit", [D, N], F32, isOutput=False)
    xht_d = nc.declare_dram_parameter("xht", [D, N], F32, isOutput=False)
    w0t_d = nc.declare_dram_parameter("w0t", [4 * F, O1], F32, isOutput=False)
    w1t_d = nc.declare_dram_parameter("w1t", [4 * F, O1], F32, isOutput=False)
    wc0t_d = nc.declare_dram_parameter("wc0t", [4 * F, O2], F32, isOutput=False)
    wc1t_d = nc.declare_dram_parameter("wc1t", [4 * F, O2], F32, isOutput=False)
    b0_d = nc.declare_dram_parameter("b0", [O1, 1], F32, isOutput=False)
    b1_d = nc.declare_dram_parameter("b1", [O1, 1], F32, isOutput=False)
    bc0_d = nc.declare_dram_parameter("bc0", [O2, 1], F32, isOutput=False)
    bc1_d = nc.declare_dram_parameter("bc1", [O2, 1], F32, isOutput=False)
    id_d = nc.declare_dram_parameter("ident", [128, 128], F32, isOutput=False)
    out_d = nc.declare_dram_parameter("out", [N, D], F32, isOutput=True)

    with tile.TileContext(nc) as tc:
        with (
            tc.tile_pool(name="sb", bufs=1) as sb,
            tc.tile_pool(name="zp", bufs=4) as zp,
            tc.tile_pool(name="nmp", bufs=2) as nmp,
            tc.tile_pool(name="ph", bufs=2, space="PSUM") as ph,
            tc.tile_pool(name="pp", bufs=1, space="PSUM") as pp,
            tc.tile_pool(name="pt", bufs=2, space="PSUM") as pt,
        ):
            # ---------------- input DMAs ----------------
            a0sb = sb.tile([128, NT, N], DT, tag="a0sb")
            a1sb = sb.tile([128, NT, N], DT, tag="a1sb")
            for t in range(NT):
                nc.sync.dma_start(a0sb[:, t, :], a0t_d[t * 128 : (t + 1) * 128, :])
                nc.sync.dma_start(a1sb[:, t, :], a1t_d[t * 128 : (t + 1) * 128, :])

            w0sb = sb.tile([128, 4, O1], F32, tag="w0sb")
            w1sb = sb.tile([128, 4, O1], F32, tag="w1sb")
            wc0sb = sb.tile([128, 4, O2], F32, tag="wc0sb")
            wc1sb = sb.tile([128, 4, O2], F32, tag="wc1sb")
            nc.sync.dma_start(w0sb[:], w0t_d[:].rearrange("(f m) o -> f m o", m=4))
            nc.sync.dma_start(w1sb[:], w1t_d[:].rearrange("(f m) o -> f m o", m=4))
            nc.sync.dma_start(wc0sb[:], wc0t_d[:].rearrange("(f m) o -> f m o", m=4))
            nc.sync.dma_start(wc1sb[:], wc1t_d[:].rearrange("(f m) o -> f m o", m=4))

            b0sb = sb.tile([O1, 1], F32, tag="b0sb")
            b1sb = sb.tile([O1, 1], F32, tag="b1sb")
            bc0sb = sb.tile([O2, 1], F32, tag="bc0sb")
            bc1sb = sb.tile([O2, 1], F32, tag="bc1sb")
            nc.sync.dma_start(b0sb[:], b0_d[:])
            nc.sync.dma_start(b1sb[:], b1_d[:])
            nc.sync.dma_start(bc0sb[:], bc0_d[:])
            nc.sync.dma_start(bc1sb[:], bc1_d[:])

            xcst = sb.tile([128, NT, F], F32, tag="xcst")
            nc.sync.dma_start(
                xcst[:, :, 0:D], xi_d[:].rearrange("(t p) d -> p t d", p=128)
            )
            nc.sync.dma_start(
                xcst[:, :, D:F], xh_d[:].rearrange("(t p) d -> p t d", p=128)
            )
            xcTst = sb.tile([128, N], F32, tag="xcTst")
            nc.sync.dma_start(xcTst[0:D, :], xit_d[:])
            nc.sync.dma_start(xcTst[D:F, :], xht_d[:])
            xhT0 = sb.tile([D, N], F32, tag="xhT0")  # xh^T at base partition 0
            nc.sync.dma_start(xhT0[:], xht_d[:])
            identf = sb.tile([128, 128], F32, tag="identf")
            nc.sync.dma_start(identf[:], id_d[:])

            # ---------------- small prep (DVE) ----------------
            ident = sb.tile([128, 128], DT, tag="ident")
            nc.vector.tensor_copy(ident[:], identf[:])
            xc = sb.tile([128, NT, F], DT, tag="xc")
            nc.vector.tensor_copy(xc[:], xcst[:])
            xcT = sb.tile([128, N], DT, tag="xcT")
            nc.vector.tensor_copy(xcT[:], xcTst[:])

            wx1 = sb.tile([128, O1], DT, tag="wx1")
            wz10 = sb.tile([128, O1], DT, tag="wz10")
            wz20 = sb.tile([128, O1], DT, tag="wz20")
            wz11 = sb.tile([128, O1], DT, tag="wz11")
            wz21 = sb.tile([128, O1], DT, tag="wz21")
            nc.vector.tensor_add(wx1[:], w0sb[:, 0, :], w1sb[:, 0, :])
            nc.vector.tensor_add(wz10[:], w0sb[:, 1, :], w0sb[:, 2, :])
            nc.vector.tensor_copy(wz20[:], w0sb[:, 3, :])
            nc.vector.tensor_add(wz11[:], w1sb[:, 1, :], w1sb[:, 2, :])
            nc.vector.tensor_copy(wz21[:], w1sb[:, 3, :])
            vx1 = sb.tile([128, O2], DT, tag="vx1")
            vz10 = sb.tile([128, O2], DT, tag="vz10")
            vz20 = sb.tile([128, O2], DT, tag="vz20")
            vz11 = sb.tile([128, O2], DT, tag="vz11")
            vz21 = sb.tile([128, O2], DT, tag="vz21")
            nc.vector.tensor_add(vx1[:], wc0sb[:, 0, :], wc1sb[:, 0, :])
            nc.vector.tensor_add(vz10[:], wc0sb[:, 1, :], wc0sb[:, 2, :])
            nc.vector.tensor_copy(vz20[:], wc0sb[:, 3, :])
            nc.vector.tensor_add(vz11[:], wc1sb[:, 1, :], wc1sb[:, 2, :])
            nc.vector.tensor_copy(vz21[:], wc1sb[:, 3, :])
            bias1 = sb.tile([O1, 1], F32, tag="bias1")
            bias2 = sb.tile([O2, 1], F32, tag="bias2")
            nc.vector.tensor_add(bias1[:], b0sb[:], b1sb[:])
            nc.vector.tensor_add(bias2[:], bc0sb[:], bc1sb[:])

            onesf = sb.tile([128, 128], F32, tag="onesf")
            nc.gpsimd.memset(onesf[:], 1.0)
            ones = sb.tile([128, 128], DT, tag="ones")
            nc.vector.tensor_copy(ones[:], onesf[:])
            ones_c = ones[:, 0:1]
            ones_r = ones[0:1, :]

            # ---------------- degree sums (PE) ----------------
            # d1[i] = sum_j a0t[j,i] = rowsum(adj+I)[i]  -> scales chain 1
            # d0[i] = sum_j a1t[j,i] = colsum(adj+I)[i]  -> scales chain 0
            dsum_a = ph.tile([128, N], F32, tag="hop")
            dsum_b = ph.tile([128, N], F32, tag="hop")
            for t in range(NT):
                for c0, c1 in CH:
                    nc.tensor.matmul(
                        dsum_a[0:1, c0:c1], ones_c, a0sb[:, t, c0:c1],
                        start=(t == 0), stop=(t == NT - 1),
                    )
            for t in range(NT):
                for c0, c1 in CH:
                    nc.tensor.matmul(
                        dsum_b[0:1, c0:c1], ones_c, a1sb[:, t, c0:c1],
                        start=(t == 0), stop=(t == NT - 1),
                    )
            d1inv = sb.tile([1, N], F32, tag="d1inv")
            d0inv = sb.tile([1, N], F32, tag="d0inv")
            nc.vector.reciprocal(d1inv[:], dsum_a[0:1, :])
            nc.vector.reciprocal(d0inv[:], dsum_b[0:1, :])
            d1inv_dt = sb.tile([1, N], DT, tag="d1inv_dt")
            d0inv_dt = sb.tile([1, N], DT, tag="d0inv_dt")
            nc.vector.tensor_copy(d1inv_dt[:], d1inv[:])
            nc.vector.tensor_copy(d0inv_dt[:], d0inv[:])
            # broadcast 1/d to all 128 partitions via outer product with ones
            bc0 = sb.tile([128, N], F32, tag="bc0")   # chain-0 scale (1/d0)
            bc1 = sb.tile([128, N], F32, tag="bc1")   # chain-1 scale (1/d1)
            pbc = pp.tile([128, N], F32, tag="pre")
            for c0, c1 in CH:
                nc.tensor.matmul(pbc[:, c0:c1], ones_r, d0inv_dt[:, c0:c1],
                                 start=True, stop=True)
            nc.vector.tensor_copy(bc0[:], pbc[:])
            pbc2 = pp.tile([128, N], F32, tag="pre")
            for c0, c1 in CH:
                nc.tensor.matmul(pbc2[:, c0:c1], ones_r, d1inv_dt[:, c0:c1],
                                 start=True, stop=True)
            nc.vector.tensor_copy(bc1[:], pbc2[:])

            def hop(act_nm, asb, scale):
                """z^T[f,i] = sum_j act[j,f] * asb[j,i], scaled per-col: -> SBUF DT."""
                ps = ph.tile([128, N], F32, tag="hop")
                for t in range(NT):
                    for c0, c1 in CH:
                        nc.tensor.matmul(
                            ps[:, c0:c1], act_nm[:, t, :], asb[:, t, c0:c1],
                            start=(t == 0), stop=(t == NT - 1),
                        )
                z = zp.tile([128, N], DT, tag="z")
                nc.vector.tensor_mul(z[:], ps[:], scale[:])
                return z

            def transpose_set(src, dt, o_parts=128):
                """src [o_parts, N] -> node-major [128, NT, o_parts] in dtype dt."""
                dst = nmp.tile([128, NT, o_parts], dt, tag=f"nm{o_parts}")
                idap = identf if dt == F32 else ident
                for t in range(NT):
                    ptr = pt.tile([128, 128], dt, tag="tr")
                    nc.tensor.transpose(
                        ptr[0:128, 0:o_parts],
                        src[:, t * 128 : (t + 1) * 128],
                        idap[0:o_parts, 0:o_parts],
                    )
                    nc.vector.tensor_copy(dst[:, t, :], ptr[0:128, 0:o_parts])
                return dst

            def gconv(act_nm, act_T, ws, psum_parts, bias):
                wx, wa0, wb0, wa1, wb1 = ws
                z1c0 = hop(act_nm, a0sb, bc0)
                z1c1 = hop(act_nm, a1sb, bc1)
                z1c0_nm = transpose_set(z1c0, DT)
                z1c1_nm = transpose_set(z1c1, DT)
                z2c0 = hop(z1c0_nm, a0sb, bc0)
                z2c1 = hop(z1c1_nm, a1sb, bc1)
                pre = pp.tile([psum_parts, N], F32, tag="pre")
                terms = [(wx, act_T), (wa0, z1c0), (wb0, z2c0), (wa1, z1c1), (wb1, z2c1)]
                for k, (w, rhs) in enumerate(terms):
                    for c0, c1 in CH:
                        nc.tensor.matmul(
                            pre[:, c0:c1], w[:], rhs[:, c0:c1],
                            start=(k == 0), stop=(k == len(terms) - 1),
                        )
                return pre

            # ---------------- gconv 1 ----------------
            pre1 = gconv(xc, xcT, (wx1, wz10, wz20, wz11, wz21), O1, bias1)
            # split sigmoid into r/u halves so downstream tensor-tensor ops see
            # matching base partitions (HW: both SB inputs must share base)
            Sig = mybir.ActivationFunctionType.Sigmoid
            val_r = sb.tile([D, N], F32, tag="val_r")
            val_u = sb.tile([D, N], F32, tag="val_u")
            nc.scalar.activation(val_r[:], pre1[0:D, :], Sig, bias=bias1[0:D, :])
            nc.scalar.activation(val_u[:], pre1[D:F, :], Sig, bias=bias1[D:F, :])

            # xc2^T = [xi^T ; (r*xh)^T]
            xc2T = sb.tile([128, N], DT, tag="xc2T")
            nc.vector.tensor_copy(xc2T[0:D, :], xcT[0:D, :])
            nc.vector.tensor_mul(xc2T[D:F, :], val_r[:], xhT0[:])
            xc2 = transpose_set(xc2T, DT)

            # ---------------- gconv 2 ----------------
            pre2 = gconv(xc2, xc2T, (vx1, vz10, vz20, vz11, vz21), O2, bias2)
            cT = sb.tile([O2, N], F32, tag="cT")
            nc.scalar.activation(
                cT[:], pre2[:], mybir.ActivationFunctionType.Tanh, bias=bias2[:]
            )

            # ---------------- combine: out = u*xh + (1-u)*c ----------------
            outT = sb.tile([O2, N], F32, tag="outT")
            nc.vector.tensor_sub(outT[:], xhT0[:], cT[:])             # xh - c
            nc.vector.tensor_mul(outT[:], val_u[:], outT[:])          # u*(xh-c)
            nc.vector.tensor_add(outT[:], outT[:], cT[:])             # + c
            out_nm = transpose_set(outT, F32, o_parts=O2)
            nc.sync.dma_start(
                out_d[:].rearrange("(t p) d -> p t d", p=128), out_nm[:]
            )

    nc.finalize()
    return nc


def _prep_inputs(inputs, hx, adj, W0, b0, W1, b1, Wc0, bc0, Wc1, bc1):
    """Host-side layout prep -> per-core input maps (no math beyond adj + I)."""
    eye = np.eye(N, dtype=np.float32)
    ident = np.eye(128, dtype=np.float32)
    shared = {
        "w0t": np.ascontiguousarray(W0.T).astype(np.float32),
        "w1t": np.ascontiguousarray(W1.T).astype(np.float32),
        "wc0t": np.ascontiguousarray(Wc0.T).astype(np.float32),
        "wc1t": np.ascontiguousarray(Wc1.T).astype(np.float32),
        "b0": b0.reshape(O1, 1).astype(np.float32),
        "b1": b1.reshape(O1, 1).astype(np.float32),
        "bc0": bc0.reshape(O2, 1).astype(np.float32),
        "bc1": bc1.reshape(O2, 1).astype(np.float32),
        "ident": ident,
    }
    in_maps = []
    xi_all = inputs.reshape(B, N, D)
    xh_all = hx.reshape(B, N, D)
    for b in range(B):
        adjI = adj[b] + eye
        m = dict(shared)
        m["a1t"] = np.ascontiguousarray(adjI).astype(NPDT)
        m["a0t"] = np.ascontiguousarray(adjI.T).astype(NPDT)
        m["xi"] = np.ascontiguousarray(xi_all[b]).astype(np.float32)
        m["xh"] = np.ascontiguousarray(xh_all[b]).astype(np.float32)
        m["xit"] = np.ascontiguousarray(xi_all[b].T).astype(np.float32)
        m["xht"] = np.ascontiguousarray(xh_all[b].T).astype(np.float32)
        in_maps.append(m)
    return in_maps


def kernel(**inputs) -> np.ndarray:
    args = {k: np.asarray(v) for k, v in inputs.items()}
    if "nc" not in _cache:
        _cache["nc"] = _build_nc()
    nc = _cache["nc"]
    in_maps = _prep_inputs(
        args["inputs"], args["hx"], args["adj"],
        args["W0"], args["b0"], args["W1"], args["b1"],
        args["Wc0"], args["bc0"], args["Wc1"], args["bc1"],
    )
    res = run_bass_kernel_spmd(nc, in_maps, list(range(B)))
    out = np.stack([res.results[b]["out"].reshape(N * D) for b in range(B)])
    return out.astype(np.float32)


# revision 9
# speedup vs baseline: 1.1234x; 1.1234x over previous
"""DRGRU (diffusion-conv GRU cell) Trainium2 kernel.

Per-core (8 cores, one batch sample each):
  A0 = diag(1/colsum(adj+I)) @ (adj+I),  A1 = diag(1/colsum(adj^T+I)) @ (adj^T+I)
  gconv(x) = [x, A0 x, A0 x, A0^2 x] @ W0^T + [x, A1 x, A1 x, A1^2 x] @ W1^T + b
  value = sigmoid(gconv1(cat(xi, xh)));  r,u = split(value)
  c = tanh(gconv2(cat(xi, r*xh)));  out = u*xh + (1-u)*c

Device layout strategy: diffusion hops compute z^T = act.T @ adjI_or[j,i]
(stationary = activation node-tile, moving = un-normalized adjacency), with the
row-normalization 1/d applied as a per-output-column scale folded into the
PSUM->SBUF copy (d is computed on device via ones-vector matmuls + reciprocal +
outer-product broadcast).  Projections contract features with small stationary
weights against feat-major activations.  PE transposes flip layouts between
hops.  All matmul operands are float32r (TF32-like, full PE rate at >=256 free).
"""

import os

import numpy as np
import ml_dtypes

import concourse.bacc as bacc
import concourse.mybir as mybir
from concourse import tile
from concourse.bass_utils import run_bass_kernel_spmd

B, N, D = 8, 1024, 64
F = 2 * D       # 128 per-node features into gconv1
NT = N // 128   # 8 node tiles
O1, O2 = 2 * D, D

F32 = mybir.dt.float32

_DT_NAME = os.environ.get("DRGRU_DT", "f32r")
if _DT_NAME == "bf16":
    DT, NPDT, FREE = mybir.dt.bfloat16, ml_dtypes.bfloat16, 512
else:
    DT, NPDT, FREE = mybir.dt.float32r, np.float32, 512
CH = [(i, min(i + FREE, N)) for i in range(0, N, FREE)]

_cache: dict = {}


def _build_nc():
    nc = bacc.Bacc("TRN2", target_bir_lowering=False, debug=False, num_devices=8)

    a0t_d = nc.declare_dram_parameter("a0t", [N, N], DT, isOutput=False)
    a1t_d = nc.declare_dram_parameter("a1t", [N, N], DT, isOutput=False)
    xi_d = nc.declare_dram_parameter("xi", [N, D], F32, isOutput=False)
    xh_d = nc.declare_dram_parameter("xh", [N, D], F32, isOutput=False)
    xit_d = nc.declare_dram_parameter("xit", [D, N], F32, isOutput=False)
    xht_d = nc.declare_dram_parameter("xht", [D, N], F32, isOutput=False)
    w0t_d = nc.declare_dram_parameter("w0t", [4 * F, O1], F32, isOutput=False)
    w1t_d = nc.declare_dram_parameter("w1t", [4 * F, O1], F32, isOutput=False)
    wc0t_d = nc.declare_dram_parameter("wc0t", [4 * F, O2], F32, isOutput=False)
    wc1t_d = nc.declare_dram_parameter("wc1t", [4 * F, O2], F32, isOutput=False)
    b0_d = nc.declare_dram_parameter("b0", [O1, 1], F32, isOutput=False)
    b1_d = nc.declare_dram_parameter("b1", [O1, 1], F32, isOutput=False)
    bc0_d = nc.declare_dram_parameter("bc0", [O2, 1], F32, isOutput=False)
    bc1_d = nc.declare_dram_parameter("bc1", [O2, 1], F32, isOutput=False)
    id_d = nc.declare_dram_parameter("ident", [128, 128], F32, isOutput=False)
    out_d = nc.declare_dram_parameter("out", [N, D], F32, isOutput=True)

    with tile.TileContext(nc) as tc:
        with (
            tc.tile_pool(name="sb", bufs=1) as sb,
            tc.tile_pool(name="zp", bufs=4) as zp,
            tc.tile_pool(name="nmp", bufs=2) as nmp,
            tc.tile_pool(name="ph", bufs=2, space="PSUM") as ph,
            tc.tile_pool(name="pp", bufs=1, space="PSUM") as pp,
            tc.tile_pool(name="pt", bufs=2, space="PSUM") as pt,
        ):
            # ---------------- input DMAs ----------------
            a0sb = sb.tile([128, NT, N], DT, tag="a0sb")
            a1sb = sb.tile([128, NT, N], DT, tag="a1sb")
            for t in range(NT):
                nc.sync.dma_start(a0sb[:, t, :], a0t_d[t * 128 : (t + 1) * 128, :])
                nc.sync.dma_start(a1sb[:, t, :], a1t_d[t * 128 : (t + 1) * 128, :])

            w0sb = sb.tile([128, 4, O1], F32, tag="w0sb")
            w1sb = sb.tile([128, 4, O1], F32, tag="w1sb")
            wc0sb = sb.tile([128, 4, O2], F32, tag="wc0sb")
            wc1sb = sb.tile([128, 4, O2], F32, tag="wc1sb")
            nc.sync.dma_start(w0sb[:], w0t_d[:].rearrange("(f m) o -> f m o", m=4))
            nc.sync.dma_start(w1sb[:], w1t_d[:].rearrange("(f m) o -> f m o", m=4))
            nc.sync.dma_start(wc0sb[:], wc0t_d[:].rearrange("(f m) o -> f m o", m=4))
            nc.sync.dma_start(wc1sb[:], wc1t_d[:].rearrange("(f m) o -> f m o", m=4))

            b0sb = sb.tile([O1, 1], F32, tag="b0sb")
            b1sb = sb.tile([O1, 1], F32, tag="b1sb")
            bc0sb = sb.tile([O2, 1], F32, tag="bc0sb")
            bc1sb = sb.tile([O2, 1], F32, tag="bc1sb")
            nc.sync.dma_start(b0sb[:], b0_d[:])
            nc.sync.dma_start(b1sb[:], b1_d[:])
            nc.sync.dma_start(bc0sb[:], bc0_d[:])
            nc.sync.dma_start(bc1sb[:], bc1_d[:])

            xcst = sb.tile([128, NT, F], F32, tag="xcst")
            nc.sync.dma_start(
                xcst[:, :, 0:D], xi_d[:].rearrange("(t p) d -> p t d", p=128)
            )
            nc.sync.dma_start(
                xcst[:, :, D:F], xh_d[:].rearrange("(t p) d -> p t d", p=128)
            )
            xcTst = sb.tile([128, N], F32, tag="xcTst")
            nc.sync.dma_start(xcTst[0:D, :], xit_d[:])
            nc.sync.dma_start(xcTst[D:F, :], xht_d[:])
            xhT0 = sb.tile([D, N], F32, tag="xhT0")  # xh^T at base partition 0
            nc.sync.dma_start(xhT0[:], xht_d[:])
            identf = sb.tile([128, 128], F32, tag="identf")
            nc.sync.dma_start(identf[:], id_d[:])

            # ---------------- small prep (DVE) ----------------
            ident = sb.tile([128, 128], DT, tag="ident")
            nc.vector.tensor_copy(ident[:], identf[:])
            xc = sb.tile([128, NT, F], DT, tag="xc")
            nc.vector.tensor_copy(xc[:], xcst[:])
            xcT = sb.tile([128, N], DT, tag="xcT")
            nc.vector.tensor_copy(xcT[:], xcTst[:])

            wx1 = sb.tile([128, O1], DT, tag="wx1")
            wz10 = sb.tile([128, O1], DT, tag="wz10")
            wz20 = sb.tile([128, O1], DT, tag="wz20")
            wz11 = sb.tile([128, O1], DT, tag="wz11")
            wz21 = sb.tile([128, O1], DT, tag="wz21")
            nc.vector.tensor_add(wx1[:], w0sb[:, 0, :], w1sb[:, 0, :])
            nc.vector.tensor_add(wz10[:], w0sb[:, 1, :], w0sb[:, 2, :])
            nc.vector.tensor_copy(wz20[:], w0sb[:, 3, :])
            nc.vector.tensor_add(wz11[:], w1sb[:, 1, :], w1sb[:, 2, :])
            nc.vector.tensor_copy(wz21[:], w1sb[:, 3, :])
            vx1 = sb.tile([128, O2], DT, tag="vx1")
            vz10 = sb.tile([128, O2], DT, tag="vz10")
            vz20 = sb.tile([128, O2], DT, tag="vz20")
            vz11 = sb.tile([128, O2], DT, tag="vz11")
            vz21 = sb.tile([128, O2], DT, tag="vz21")
            nc.vector.tensor_add(vx1[:], wc0sb[:, 0, :], wc1sb[:, 0, :])
            nc.vector.tensor_add(vz10[:], wc0sb[:, 1, :], wc0sb[:, 2, :])
            nc.vector.tensor_copy(vz20[:], wc0sb[:, 3, :])
            nc.vector.tensor_add(vz11[:], wc1sb[:, 1, :], wc1sb[:, 2, :])
            nc.vector.tensor_copy(vz21[:], wc1sb[:, 3, :])
            bias1 = sb.tile([O1, 1], F32, tag="bias1")
            bias2 = sb.tile([O2, 1], F32, tag="bias2")
            nc.vector.tensor_add(bias1[:], b0sb[:], b1sb[:])
            nc.vector.tensor_add(bias2[:], bc0sb[:], bc1sb[:])

            onesf = sb.tile([128, 128], F32, tag="onesf")
            nc.gpsimd.memset(onesf[:], 1.0)
            ones = sb.tile([128, 128], DT, tag="ones")
            nc.vector.tensor_copy(ones[:], onesf[:])
            ones_c = ones[:, 0:1]
            ones_r = ones[0:1, :]

            # ---------------- degree sums (PE) ----------------
            # d1[i] = sum_j a0t[j,i] = rowsum(adj+I)[i]  -> scales chain 1
            # d0[i] = sum_j a1t[j,i] = colsum(adj+I)[i]  -> scales chain 0
            dsum_a = ph.tile([128, N], F32, tag="hop")
            dsum_b = ph.tile([128, N], F32, tag="hop")
            for t in range(NT):
                for c0, c1 in CH:
                    nc.tensor.matmul(
                        dsum_a[0:1, c0:c1], ones_c, a0sb[:, t, c0:c1],
                        start=(t == 0), stop=(t == NT - 1),
                    )
            for t in range(NT):
                for c0, c1 in CH:
                    nc.tensor.matmul(
                        dsum_b[0:1, c0:c1], ones_c, a1sb[:, t, c0:c1],
                        start=(t == 0), stop=(t == NT - 1),
                    )
            d1inv = sb.tile([1, N], F32, tag="d1inv")
            d0inv = sb.tile([1, N], F32, tag="d0inv")
            nc.vector.reciprocal(d1inv[:], dsum_a[0:1, :])
            nc.vector.reciprocal(d0inv[:], dsum_b[0:1, :])
            d1inv_dt = sb.tile([1, N], DT, tag="d1inv_dt")
            d0inv_dt = sb.tile([1, N], DT, tag="d0inv_dt")
            nc.vector.tensor_copy(d1inv_dt[:], d1inv[:])
            nc.vector.tensor_copy(d0inv_dt[:], d0inv[:])
            # broadcast 1/d to all 128 partitions via outer product with ones
            bc0 = sb.tile([128, N], F32, tag="bc0")   # chain-0 scale (1/d0)
            bc1 = sb.tile([128, N], F32, tag="bc1")   # chain-1 scale (1/d1)
            pbc = pp.tile([128, N], F32, tag="pre")
            for c0, c1 in CH:
                nc.tensor.matmul(pbc[:, c0:c1], ones_r, d0inv_dt[:, c0:c1],
                                 start=True, stop=True)
            nc.vector.tensor_copy(bc0[:], pbc[:])
            pbc2 = pp.tile([128, N], F32, tag="pre")
            for c0, c1 in CH:
                nc.tensor.matmul(pbc2[:, c0:c1], ones_r, d1inv_dt[:, c0:c1],
                                 start=True, stop=True)
            nc.vector.tensor_copy(bc1[:], pbc2[:])

            def hop(act_nm, asb, scale):
                """z^T[f,i] = sum_j act[j,f] * asb[j,i], scaled per-col: -> SBUF DT."""
                ps = ph.tile([128, N], F32, tag="hop")
                for t in range(NT):
                    for c0, c1 in CH:
                        nc.tensor.matmul(
                            ps[:, c0:c1], act_nm[:, t, :], asb[:, t, c0:c1],
                            start=(t == 0), stop=(t == NT - 1),
                        )
                z = zp.tile([128, N], DT, tag="z")
                nc.vector.tensor_mul(z[:], ps[:], scale[:])
                return z

            def transpose_set(src, dt, o_parts=128):
                """src [o_parts, N] -> node-major [128, NT, o_parts] in dtype dt."""
                dst = nmp.tile([128, NT, o_parts], dt, tag=f"nm{o_parts}")
                idap = identf if dt == F32 else ident
                for t in range(NT):
                    ptr = pt.tile([128, 128], dt, tag="tr")
                    nc.tensor.transpose(
                        ptr[0:128, 0:o_parts],
                        src[:, t * 128 : (t + 1) * 128],
                        idap[0:o_parts, 0:o_parts],
                    )
                    nc.vector.tensor_copy(dst[:, t, :], ptr[0:128, 0:o_parts])
                return dst

            def gconv(act_nm, act_T, ws, psum_parts, bias):
                wx, wa0, wb0, wa1, wb1 = ws
                z1c0 = hop(act_nm, a0sb, bc0)
                z1c1 = hop(act_nm, a1sb, bc1)
                z1c0_nm = transpose_set(z1c0, DT)
                z1c1_nm = transpose_set(z1c1, DT)
                z2c0 = hop(z1c0_nm, a0sb, bc0)
                z2c1 = hop(z1c1_nm, a1sb, bc1)
                pre = pp.tile([psum_parts, N], F32, tag="pre")
                terms = [(wx, act_T), (wa0, z1c0), (wb0, z2c0), (wa1, z1c1), (wb1, z2c1)]
                for k, (w, rhs) in enumerate(terms):
                    for c0, c1 in CH:
                        nc.tensor.matmul(
                            pre[:, c0:c1], w[:], rhs[:, c0:c1],
                            start=(k == 0), stop=(k == len(terms) - 1),
                        )
                return pre

            # ---------------- gconv 1 ----------------
            pre1 = gconv(xc, xcT, (wx1, wz10, wz20, wz11, wz21), O1, bias1)
            # split sigmoid into r/u halves so downstream tensor-tensor ops see
            # matching base partitions (HW: both SB inputs must share base)
            Sig = mybir.ActivationFunctionType.Sigmoid
            val_r = sb.tile([D, N], F32, tag="val_r")
            val_u = sb.tile([D, N], F32, tag="val_u")
            nc.scalar.activation(val_r[:], pre1[0:D, :], Sig, bias=bias1[0:D, :])
            nc.scalar.activation(val_u[:], pre1[D:F, :], Sig, bias=bias1[D:F, :])

            # xc2^T = [xi^T ; (r*xh)^T]
            xc2T = sb.tile([128, N], DT, tag="xc2T")
            nc.vector.tensor_copy(xc2T[0:D, :], xcT[0:D, :])
            nc.vector.tensor_mul(xc2T[D:F, :], val_r[:], xhT0[:])
            xc2 = transpose_set(xc2T, DT)

            # ---------------- gconv 2 ----------------
            pre2 = gconv(xc2, xc2T, (vx1, vz10, vz20, vz11, vz21), O2, bias2)
            cT = sb.tile([O2, N], F32, tag="cT")
            nc.scalar.activation(
                cT[:], pre2[:], mybir.ActivationFunctionType.Tanh, bias=bias2[:]
            )

            # ---------------- combine: out = u*xh + (1-u)*c ----------------
            outT = sb.tile([O2, N], F32, tag="outT")
            nc.vector.tensor_sub(outT[:], xhT0[:], cT[:])             # xh - c
            nc.vector.tensor_mul(outT[:], val_u[:], outT[:])          # u*(xh-c)
            nc.vector.tensor_add(outT[:], outT[:], cT[:])             # + c
            out_nm = transpose_set(outT, F32, o_parts=O2)
            nc.sync.dma_start(
                out_d[:].rearrange("(t p) d -> p t d", p=128), out_nm[:]
            )

    nc.finalize()
    return nc


def _prep_inputs(inputs, hx, adj, W0, b0, W1, b1, Wc0, bc0, Wc1, bc1):
    """Host-side layout prep -> per-core input maps (no math beyond adj + I)."""
    eye = np.eye(N, dtype=np.float32)
    ident = np.eye(128, dtype=np.float32)
    shared = {
        "w0t": np.ascontiguousarray(W0.T).astype(np.float32),
        "w1t": np.ascontiguousarray(W1.T).astype(np.float32),
        "wc0t": np.ascontiguousarray(Wc0.T).astype(np.float32),
        "wc1t": np.ascontiguousarray(Wc1.T).astype(np.float32),
        "b0": b0.reshape(O1, 1).astype(np.float32),
        "b1": b1.reshape(O1, 1).astype(np.float32),
        "bc0": bc0.reshape(O2, 1).astype(np.float32),
        "bc1": bc1.reshape(O2, 1).astype(np.float32),
        "ident": ident,
    }
    in_maps = []
    xi_all = inputs.reshape(B, N, D)
    xh_all = hx.reshape(B, N, D)
    for b in range(B):
        adjI = adj[b] + eye
        m = dict(shared)
        m["a1t"] = np.ascontiguousarray(adjI).astype(NPDT)
        m["a0t"] = np.ascontiguousarray(adjI.T).astype(NPDT)
        m["xi"] = np.ascontiguousarray(xi_all[b]).astype(np.float32)
        m["xh"] = np.ascontiguousarray(xh_all[b]).astype(np.float32)
        m["xit"] = np.ascontiguousarray(xi_all[b].T).astype(np.float32)
        m["xht"] = np.ascontiguousarray(xh_all[b].T).astype(np.float32)
        in_maps.append(m)
    return in_maps


def kernel(**inputs) -> np.ndarray:
    args = {k: np.asarray(v) for k, v in inputs.items()}
    if "nc" not in _cache:
        _cache["nc"] = _build_nc()
    nc = _cache["nc"]
    in_maps = _prep_inputs(
        args["inputs"], args["hx"], args["adj"],
        args["W0"], args["b0"], args["W1"], args["b1"],
        args["Wc0"], args["bc0"], args["Wc1"], args["bc1"],
    )
    res = run_bass_kernel_spmd(nc, in_maps, list(range(B)))
    out = np.stack([res.results[b]["out"].reshape(N * D) for b in range(B)])
    return out.astype(np.float32)
